# revision 1
# baseline (speedup 1.0000x reference)
"""Trainium2 Bass kernel for nn_Attention_Block (quirky reshape + axis-2 softmax).

Reference math (B=4, T=2048, D=512, H=8, hd=64):
  q = x @ Wq.T ; k = x @ Wk.T ; v = x @ Wv.T          (per batch, [T, D])
  q/k/v reshaped RAW to [H, T, hd]  -> head h == contiguous 256-row chunk of
  the [T, D] matrix, reinterpreted as [2048, 64].
  scores = q~ @ k~.T / 8 ; attn = softmax(scores, axis=q) ; out = attn @ v~
  reshaped back, then @ Wo.T + bo.

Because the head split is a raw reshape, the whole problem decomposes into
B*H = 32 independent 256-row units.  We run 8-way data parallel (4 units per
core, weights replicated, no collectives).

Per-unit kernel layout (core insight: with S^T = k~ @ q~.T the softmax over q
becomes a row softmax along the free axis):
  - permuted ordering q' = (j, r): q~'^T block j = rows 64j..64j+63 of
    QT = Wq @ x_u^T (QT tiles natively hold two blocks per 128 partitions);
    qt_swp (partition halves exchanged, via SBUF->SBUF DMA) makes all four
    blocks of either parity available at either partition half
  - projections/final matmuls in float32r (full-rate fp32; plain fp32 is
    1/4 rate), attention matmuls in bf16, S^T row-tiled via tile_position
  - exp on ScalarE (scale=1/8 folded in, accum_out gives Z; no max
    subtraction needed: |scores/8| < ~1.2 for these inputs), 1/Z folded
    into the v chunk; a few chunks per unit use a one-instruction
    Schraudolph bf16 fast-exp on VectorE to offload ScalarE
  - PV col-tiled 2x producing out^T directly in the layout that makes
    OC^T (the final-projection lhsT) a set of aligned psum->sbuf copies
  - software-pipelined emission: proj(u+1) ahead of chunks(u), deferred
    prologue transposes backfill PE/DVE during the ACT-bound chunk loop
    (writes always emitted before their readers - Tile tracks deps in
    emission order).
"""

import numpy as np

D = 512
TCORE = 1024  # rows of x per core
NU = 4        # units (b,h pairs) per core
NCORES = 8

_CHUNK_ORDER = [
    (0, 0), (1, 0), (0, 1), (1, 1),
    (2, 0), (3, 0), (2, 1), (3, 1),
    (4, 0), (5, 0), (4, 1), (5, 1),
    (6, 0), (7, 0), (6, 1), (7, 1),
]

# Schraudolph bf16 fast-exp constants: exp(s/8) bits ~ int16(A*s + B)
_SCH_A = 128.0 / 0.6931471805599453 * 0.125
_SCH_B = 16250.25
# chunks whose exp runs on DVE instead of ScalarE (whole chunks so the
# approximation bias cancels in P/Z)
_DVE_CHUNKS_BY_UNIT = {1: {13}, 2: {3, 5, 7, 9, 11, 13}, 3: {3, 5, 7, 9, 11, 13}}
_NO_SCOPY = True
_DVE_MIXED = True
_PPOOL_BUFS = 10
_WARM = 8
_UNITP_BUFS = 3

_nc_cache = None


def _build_nc():
    from contextlib import ExitStack

    import concourse.bass as bass
    import concourse.bacc as bacc
    import concourse.mybir as mybir
    import concourse.tile as tile
    from concourse.masks import make_identity

    F32 = mybir.dt.float32
    R32 = mybir.dt.float32r
    BF16 = mybir.dt.bfloat16
    I16 = mybir.dt.int16
    EXP = mybir.ActivationFunctionType.Exp

    nc = bacc.Bacc()
    x_d = nc.dram_tensor("x", [TCORE, D], F32, kind="ExternalInput")
    w_d = {
        nm: nc.dram_tensor(nm, [D, D], F32, kind="ExternalInput")
        for nm in ("Wq", "Wk", "Wv", "Wo")
    }
    bo_d = nc.dram_tensor("bo", [D], F32, kind="ExternalInput")
    out_d = nc.dram_tensor("out", [TCORE, D], F32, kind="ExternalOutput")

    with tile.TileContext(nc) as tc, ExitStack() as ctx:
        const = ctx.enter_context(tc.tile_pool(name="const", bufs=1))
        wload = ctx.enter_context(tc.tile_pool(name="wload", bufs=1))
        unitp = ctx.enter_context(tc.tile_pool(name="unitp", bufs=_UNITP_BUFS))
        ppool = ctx.enter_context(tc.tile_pool(name="ppool", bufs=_PPOOL_BUFS))
        stats = ctx.enter_context(tc.tile_pool(name="stats", bufs=24))
        outp = ctx.enter_context(tc.tile_pool(name="outp", bufs=3))
        ps_s = ctx.enter_context(tc.tile_pool(name="ps_s", bufs=2, space="PSUM"))
        ps_o = ctx.enter_context(tc.tile_pool(name="ps_o", bufs=1, space="PSUM"))
        ps_m = ctx.enter_context(tc.tile_pool(name="ps_m", bufs=2, space="PSUM"))

        ident = const.tile([128, 128], F32, tag="ident")
        make_identity(nc, ident)

        x_sb = const.tile([128, 8, D], F32, tag="x_sb")
        xT = const.tile([128, 4, TCORE], R32, tag="xT")
        wT = {}

        def _vcopy(out, in_):
            nc.vector.tensor_copy(out=out, in_=in_)

        def _scopy(out, in_):
            nc.scalar.copy(out=out, in_=in_)

        copy_eng = [_vcopy, _vcopy] if _NO_SCOPY else [_vcopy, _scopy]
        w_sbs = {}

        def load_w_dma(nm, nsplit=2):
            w_sb = wload.tile([128, 4, D], F32, tag=f"wsb_{nm}")
            # split DMAs so the first row-tiles' transposes can start earlier
            ap = w_d[nm][:, :].rearrange("(t p) d -> p t d", p=128)
            step = 4 // nsplit
            for s in range(nsplit):
                nc.sync.dma_start(out=w_sb[:, s * step:(s + 1) * step, :],
                                  in_=ap[:, s * step:(s + 1) * step, :])
            w_sbs[nm] = w_sb
            wt = const.tile([128, 4, D], R32, tag=f"{nm}T")
            wT[nm] = wt

        def load_w_trans(nm, ci=0):
            w_sb, wt = w_sbs[nm], wT[nm]
            for k in range(4):
                for t in range(4):
                    pst = ps_m.tile([128, 2, 256], F32, tag="misc")
                    nc.tensor.transpose(
                        pst[:, 0, 0:128], w_sb[:, t, 128 * k:128 * k + 128], ident
                    )
                    eng = _vcopy if ci < 0 else copy_eng[(ci + k + t) % 2]
                    eng(wt[:, k, 128 * t:128 * t + 128], pst[:, 0, 0:128])

        def trans_x(trange, ci=0):
            for k in range(4):
                for t in trange:
                    pst = ps_m.tile([128, 2, 256], F32, tag="misc")
                    nc.tensor.transpose(pst[:, 0, 0:128], x_sb[:, t, 128 * k:128 * k + 128], ident)
                    eng = _vcopy if ci < 0 else copy_eng[(ci + k + t) % 2]
                    eng(xT[:, k, 128 * t:128 * t + 128], pst[:, 0, 0:128])

        # prologue: all DMAs early; only unit-0-critical transposes up front
        load_w_dma("Wq", nsplit=4)
        x_ap = x_d[:, :].rearrange("(t p) d -> p t d", p=128)
        for s in range(4):
            nc.sync.dma_start(out=x_sb[:, 2 * s:2 * s + 2, :],
                              in_=x_ap[:, 2 * s:2 * s + 2, :])
        load_w_dma("Wk", nsplit=4)
        load_w_dma("Wv")
        load_w_dma("Wo")
        bo_bc = const.tile([128, D], F32, tag="bo")
        nc.sync.dma_start(
            out=bo_bc, in_=bass.AP(tensor=bo_d, offset=0, ap=[[0, 128], [1, D]])
        )
        load_w_trans("Wq", 0)
        trans_x([0, 1], 0)
        load_w_trans("Wk", 1)

        def emit_proj_v(u):
            xTu = 256 * u
            pcopy_v = (lambda o, i_, n: copy_eng[n % 2](o, i_)) if u < 2 else (
                lambda o, i_, n: _vcopy(o, i_))
            # ---- V = x_u @ Wv^T, natural layout
            vv = unitp.tile([128, 2, 512], F32, tag="vv")
            for nt in range(2):
                psv = ps_m.tile([128, 2, 256], F32, tag="misc")
                for ki in range(4):
                    nc.tensor.matmul(
                        psv[:, 0:2, :],
                        lhsT=xT[:, ki, xTu + 128 * nt:xTu + 128 * nt + 128],
                        rhs=wT["Wv"][:, ki, :],
                        start=(ki == 0), stop=(ki == 3),
                    )
                pcopy_v(vv[:, nt, :], psv[:, 0:2, :], nt)
            return vv

        def emit_proj(u):
            # units 0/1 run while ScalarE is still idle: alternate their
            # psum evacuations between DVE and ScalarE; later units keep DVE
            pcopy = (lambda o, i_, n: copy_eng[n % 2](o, i_)) if u < 2 else (
                lambda o, i_, n: _vcopy(o, i_))
            xTu = 256 * u
            # ---- QT = Wq @ x_u^T, natural layout + half-swapped DMA copy.
            # qt_nat[64*b+d, m, r] = q~'^T block (2m+b); qt_swp has the two
            # partition halves exchanged, so at either half b0 the four
            # blocks of each parity are adjacent in the m (free) dim.
            qt_nat = unitp.tile([128, 4, 256], BF16, tag="qt_nat")
            qt_swp = unitp.tile([128, 4, 256], BF16, tag="qt_swp")
            for mt in range(4):
                psq = ps_m.tile([128, 2, 256], F32, tag="misc")
                for ki in range(4):
                    nc.tensor.matmul(
                        psq[:, 0, :],
                        lhsT=wT["Wq"][:, ki, 128 * mt:128 * mt + 128],
                        rhs=xT[:, ki, xTu:xTu + 256],
                        start=(ki == 0), stop=(ki == 3),
                    )
                pcopy(qt_nat[:, mt, :], psq[:, 0, :], mt)
            nc.sync.dma_start(out=qt_swp[0:64, :, :], in_=qt_nat[64:128, :, :])
            nc.sync.dma_start(out=qt_swp[64:128, :, :], in_=qt_nat[0:64, :, :])

            # ---- KT = Wk @ x_u^T, natural layout (block 2m | 2m+1 per tile)
            kt = unitp.tile([128, 4, 256], BF16, tag="kt")
            for mt in range(4):
                psk = ps_m.tile([128, 2, 256], F32, tag="misc")
                for ki in range(4):
                    nc.tensor.matmul(
                        psk[:, 0, :],
                        lhsT=wT["Wk"][:, ki, 128 * mt:128 * mt + 128],
                        rhs=xT[:, ki, xTu:xTu + 256],
                        start=(ki == 0), stop=(ki == 3),
                    )
                pcopy(kt[:, mt, :], psk[:, 0, :], mt + 1)

            return qt_nat, qt_swp, kt

        WARM = _WARM  # chunks of the next unit whose S+exp are emitted early

        def emit_score_exp(u, kc, tiles):
            """S^T matmuls + exp (+Z) for one chunk; PV is emitted separately."""
            qt_nat, qt_swp, kt = tiles
            jb, h = _CHUNK_ORDER[kc]
            b0 = jb % 2
            lhsT_s = kt[64 * b0:64 * b0 + 64, jb // 2, 128 * h:128 * h + 128]
            # pT group g=0: blocks j = 2m+b0 (from qt_nat);
            #    group g=1: blocks j = 2m+(1-b0) (from qt_swp).
            pT = ppool.tile([128, 2, 4, 256], BF16, tag="pT")
            use_dve = kc in _DVE_CHUNKS_BY_UNIT.get(u, set())
            rs = []
            reduces = []
            for g, qsrc in enumerate((qt_nat, qt_swp)):
                pss = ps_s.tile([128, 4, 256], F32, tag="ps_s")
                for a in range(2):
                    nc.tensor.matmul(
                        pss[:, 2 * a:2 * a + 2, :],
                        lhsT=lhsT_s,
                        rhs=qsrc[64 * b0:64 * b0 + 64, 2 * a:2 * a + 2, :],
                        start=True, stop=True,
                        tile_position=(64 * b0, 0),
                    )
                r = stats.tile([128, 1], F32, tag="rs")
                if use_dve and (g == 1 or not _DVE_MIXED):
                    # Schraudolph fast exp on DVE: bf16 bit pattern of
                    # exp(s/8) ~= int16(A*s + B); bias cancels in P/Z.
                    nc.vector.tensor_scalar(
                        out=pT[:, g, :, :].bitcast(I16),
                        in0=pss, scalar1=_SCH_A, scalar2=_SCH_B,
                        op0=mybir.AluOpType.mult, op1=mybir.AluOpType.add,
                    )
                    reduces.append((r, g))
                else:
                    nc.scalar.activation(
                        out=pT[:, g, :, :],
                        in_=pss, func=EXP, scale=0.125, accum_out=r,
                    )
                rs.append(r)
            for r, g in reduces:
                nc.vector.reduce_sum(
                    out=r, in_=pT[:, g, :, :],
                    axis=mybir.AxisListType.XY,
                )
            rz = stats.tile([128, 1], F32, tag="rz")
            nc.vector.tensor_add(out=rz, in0=rs[0], in1=rs[1])
            nc.vector.reciprocal(out=rz, in_=rz)
            return pT, rz, b0

        def emit_pv(kc, po, pT, rz, b0, vv):
            jb, h = _CHUNK_ORDER[kc]
            vs = stats.tile([128, 64], BF16, tag="vs")
            nc.vector.tensor_scalar_mul(
                out=vs, in0=vv[:, h, 64 * jb:64 * jb + 64], scalar1=rz
            )
            st, sp = (kc == 0), (kc == 15)
            g_even = b0          # group holding even j blocks
            g_odd = 1 - b0
            for e in range(2):
                nc.tensor.matmul(
                    po[0:64, 2 * e:2 * e + 2, :],
                    lhsT=vs, rhs=pT[:, g_even, 2 * e:2 * e + 2, :],
                    start=st, stop=sp, tile_position=(0, 0),
                    skip_group_check=True,
                )
                nc.tensor.matmul(
                    po[64:128, 2 * e:2 * e + 2, :],
                    lhsT=vs, rhs=pT[:, g_odd, 2 * e:2 * e + 2, :],
                    start=st, stop=sp, tile_position=(0, 64),
                    skip_group_check=True,
                )

        def emit_warmup(u, tiles):
            return [emit_score_exp(u, kc, tiles) for kc in range(WARM)]

        def emit_body(u, tiles, vv, warm, defer_fn=None):
            po = ps_o.tile([128, 4, 256], F32, tag="po")
            for kc in range(WARM):
                pT, rz, b0 = warm[kc]
                emit_pv(kc, po, pT, rz, b0, vv)
            for kc in range(WARM, 16):
                if kc == 8 and defer_fn is not None:
                    defer_fn()
                pT, rz, b0 = emit_score_exp(u, kc, tiles)
                emit_pv(kc, po, pT, rz, b0, vv)
            return po

        def emit_tail(u, po):
            # ---- OC^T evacuation (aligned) + final projection + bias
            ot = unitp.tile([128, 4, 256], R32, tag="ot")
            for i in range(4):
                nc.vector.tensor_copy(out=ot[:, i, :], in_=po[:, i, :])
            for m in range(2):
                psf = ps_m.tile([128, 2, 256], F32, tag="misc")
                for ki in range(4):
                    nc.tensor.matmul(
                        psf[:, 0:2, :],
                        lhsT=ot[:, ki, 128 * m:128 * m + 128],
                        rhs=wT["Wo"][:, ki, :],
                        start=(ki == 0), stop=(ki == 3),
                    )
                osb = outp.tile([128, D], F32, tag="osb")
                nc.vector.tensor_add(out=osb, in0=psf[:, 0:2, :].rearrange('p a r -> p (a r)'), in1=bo_bc)
                row = 256 * u + 128 * m
                nc.sync.dma_start(out=out_d[row:row + 128, :], in_=osb)

        # software pipeline: the next unit's first S+exp chunks (warmup) are
        # emitted before the current unit's tail so the ScalarE stream never
        # drains at unit boundaries; later projections and deferred prologue
        # transposes are emitted mid-body (defer hooks) so their PE/DVE work
        # backfills the ACT-bound chunk loop instead of blocking its start
        t0 = emit_proj(0)
        trans_x([2, 3], 0)
        t1 = emit_proj(1)
        w0 = emit_warmup(0, t0)
        # V path is off the first-exp critical path: emitted after warmup(0)
        load_w_trans("Wv", 0)
        v0 = emit_proj_v(0)
        v1 = emit_proj_v(1)
        po0 = emit_body(0, t0, v0, w0, defer_fn=lambda: load_w_trans("Wo", -1))
        w1 = emit_warmup(1, t1)
        # deferred x transposes: readers are proj(2)/proj(3), emitted after
        trans_x(list(range(4, 8)), -1)
        t2 = emit_proj(2)
        v2 = emit_proj_v(2)
        emit_tail(0, po0)
        po1 = emit_body(1, t1, v1, w1)
        w2 = emit_warmup(2, t2)
        t3 = emit_proj(3)
        v3 = emit_proj_v(3)
        emit_tail(1, po1)
        po2 = emit_body(2, t2, v2, w2)
        w3 = emit_warmup(3, t3)
        emit_tail(2, po2)
        po3 = emit_body(3, t3, v3, w3)
        emit_tail(3, po3)
    nc.compile()
    return nc


def _get_nc():
    global _nc_cache
    if _nc_cache is None:
        _nc_cache = _build_nc()
    return _nc_cache


def _run(inputs, trace=False):
    from concourse.bass_utils import run_bass_kernel_spmd

    emb = np.ascontiguousarray(np.asarray(inputs["embedding"], dtype=np.float32))
    x_flat = emb.reshape(NCORES * TCORE, D)
    shared = {
        nm: np.ascontiguousarray(np.asarray(inputs[nm], dtype=np.float32))
        for nm in ("Wq", "Wk", "Wv", "Wo", "bo")
    }
    in_maps = []
    for c in range(NCORES):
        m = {"x": np.ascontiguousarray(x_flat[TCORE * c:TCORE * (c + 1)])}
        m.update(shared)
        in_maps.append(m)

    nc = _get_nc()
    res = run_bass_kernel_spmd(
        nc, in_maps, core_ids=list(range(NCORES)), trace=trace
    )
    out_flat = np.concatenate([r["out"] for r in res.results], axis=0)
    out = out_flat.reshape(emb.shape)
    return out, res


def kernel(**inputs):
    out, _ = _run(inputs, trace=False)
    return out


def _make_in_maps(inputs):
    emb = np.ascontiguousarray(np.asarray(inputs["embedding"], dtype=np.float32))
    x_flat = emb.reshape(NCORES * TCORE, D)
    shared = {
        nm: np.ascontiguousarray(np.asarray(inputs[nm], dtype=np.float32))
        for nm in ("Wq", "Wk", "Wv", "Wo", "bo")
    }
    in_maps = []
    for c in range(NCORES):
        m = {"x": np.ascontiguousarray(x_flat[TCORE * c:TCORE * (c + 1)])}
        m.update(shared)
        in_maps.append(m)
    return in_maps


def bench(inputs, iters=20):
    """Wall-clock the sharded PJRT executable; returns min per-iter ns.

    Mirrors run_bass_via_pjrt but keeps the jitted fn + device inputs so
    repeated calls time only the NEFF execution + dispatch overhead.
    """
    import time

    import jax
    import concourse.mybir as mybir
    from jax.sharding import Mesh, PartitionSpec
    from jax.experimental.shard_map import shard_map
    from concourse.bass2jax import (
        _bass_exec_p,
        install_neuronx_cc_hook,
        partition_id_tensor,
    )

    install_neuronx_cc_hook()
    nc = _get_nc()
    in_maps = _make_in_maps(inputs)

    partition_name = nc.partition_id_tensor.name if nc.partition_id_tensor else None
    in_names, out_names, out_avals, zero_outs = [], [], [], []
    for alloc in nc.m.functions[0].allocations:
        if not isinstance(alloc, mybir.MemoryLocationSet):
            continue
        name = alloc.memorylocations[0].name
        if alloc.kind == "ExternalInput":
            if name != partition_name:
                in_names.append(name)
        elif alloc.kind == "ExternalOutput":
            shape = tuple(alloc.tensor_shape)
            dtype = mybir.dt.np(alloc.dtype)
            out_names.append(name)
            out_avals.append(jax.core.ShapedArray(shape, dtype))
            zero_outs.append(np.zeros(shape, dtype))
    n_params = len(in_names)
    n_outs = len(out_avals)
    all_in_names = list(in_names) + list(out_names)
    if partition_name is not None:
        all_in_names.append(partition_name)

    def _body(*args):
        operands = list(args)
        if partition_name is not None:
            operands.append(partition_id_tensor())
        outs = _bass_exec_p.bind(
            *operands,
            out_avals=tuple(out_avals),
            in_names=tuple(all_in_names),
            out_names=tuple(out_names),
            lowering_input_output_aliases=(),
            sim_require_finite=True,
            sim_require_nnan=True,
            nc=nc,
        )
        return tuple(outs)

    devices = jax.devices()[:NCORES]
    mesh = Mesh(np.asarray(devices), ("core",))
    in_specs = (PartitionSpec("core"),) * (n_params + n_outs)
    out_specs = (PartitionSpec("core"),) * len(out_names)
    sharded = jax.jit(
        shard_map(_body, mesh=mesh, in_specs=in_specs, out_specs=out_specs,
                  check_rep=False),
        keep_unused=True,
    )
    per_core = [[np.asarray(m[nm]) for nm in in_names] for m in in_maps]
    concat_in = [
        np.concatenate([per_core[c][i] for c in range(NCORES)], axis=0)
        for i in range(n_params)
    ]
    concat_zeros = [
        np.zeros((NCORES * z.shape[0], *z.shape[1:]), z.dtype) for z in zero_outs
    ]
    args = [jax.device_put(a) for a in concat_in + concat_zeros]
    out = sharded(*args)
    jax.block_until_ready(out)
    times = []
    for _ in range(iters):
        t0 = time.perf_counter()
        out = sharded(*args)
        jax.block_until_ready(out)
        times.append(time.perf_counter() - t0)
    times.sort()
    print(f"bench: min {times[0]*1e6:.0f}us  p50 {times[len(times)//2]*1e6:.0f}us  "
          f"max {times[-1]*1e6:.0f}us over {iters} iters")
    return times[0] * 1e9

